# revision 1
# baseline (speedup 1.0000x reference)
"""GuidedAttentionLoss on 8 Trainium2 NeuronCores (Bass/Tile, SPMD).

Math (per sample b, SIGMA=0.4, k=1/(2*0.4^2)=3.125):
  guided[x,y] = 1 - exp(-k*(y/il - x/ol)^2)   on the valid rect x<ol, y<il
  l1[b] = sum(guided*att)/ol ; l2[b] = sum((guided*att)^2)/ol

Device layout: one SBUF partition row = one valid output row (b, x); the
free dim is y in [0, 512).  Only rows x < ol are shipped (~50% of the
input on average); att is zero-padded beyond il so padded elements
contribute t = (1-e)*0 = 0.  Per 128x512 tile:
  ACT: d = Square(yv*scale + bias)   scale=1/il, bias=-x/ol per partition
  ACT: e = Exp(-3.125*d)
  DVE: t = (e-1)*att, accum r1 = sum_y t          (scalar_tensor_tensor)
  even tiles DVE: tsq = (t*1)*t, accum r2 = sum_y t^2
  odd  tiles ACT: tsq = Square(t),  accum r2      (engine balance)
Host aggregates r1/r2 per sample in f64: l1 = -sum(r1)/ol, l2 = sum(r2)/ol.

All sample-shape information lives in the packed input data (A, sc, bc),
so a single SPMD program (fixed tile count T) serves all 8 cores.
"""
import math
import numpy as np

N_CORES = 8
T_IN = 512
KEXP = -3.125

_cache = {}


def _build_program(T):
    import concourse.bacc as bacc
    import concourse.mybir as mybir
    import concourse.tile as tile

    F32 = mybir.dt.float32
    nc = bacc.Bacc("TRN2", target_bir_lowering=False, debug=False,
                   num_devices=1)
    A = nc.declare_dram_parameter("A", [T * 128, T_IN], F32, isOutput=False)
    yvp = nc.declare_dram_parameter("yv", [128, T_IN], F32, isOutput=False)
    scp = nc.declare_dram_parameter("sc", [128, T], F32, isOutput=False)
    bcp = nc.declare_dram_parameter("bc", [128, T], F32, isOutput=False)
    r1p = nc.declare_dram_parameter("r1", [128, T], F32, isOutput=True)
    r2p = nc.declare_dram_parameter("r2", [128, T], F32, isOutput=True)

    Sq = mybir.ActivationFunctionType.Square
    Ex = mybir.ActivationFunctionType.Exp
    sub = mybir.AluOpType.subtract
    mult = mybir.AluOpType.mult

    with tile.TileContext(nc) as tc:
        with tc.tile_pool(name="aux", bufs=1) as aux, \
             tc.tile_pool(name="p", bufs=3) as pa, \
             tc.tile_pool(name="pd", bufs=2) as pd, \
             tc.tile_pool(name="pe", bufs=2) as pe, \
             tc.tile_pool(name="pt", bufs=2) as pt, \
             tc.tile_pool(name="pq", bufs=2) as pq:
            yvt = aux.tile([128, T_IN], F32)
            nc.sync.dma_start(yvt[:], yvp[:])
            sct = aux.tile([128, T], F32)
            nc.sync.dma_start(sct[:], scp[:])
            bct = aux.tile([128, T], F32)
            nc.sync.dma_start(bct[:], bcp[:])
            r1t = aux.tile([128, T], F32)
            r2t = aux.tile([128, T], F32)
            for t_i in range(T):
                at = pa.tile([128, T_IN], F32)
                nc.sync.dma_start(at[:], A[t_i * 128:(t_i + 1) * 128, :])
                d = pd.tile([128, T_IN], F32)
                nc.scalar.activation(d[:], yvt[:], Sq,
                                     bias=bct[:, t_i:t_i + 1],
                                     scale=sct[:, t_i:t_i + 1])
                e = pe.tile([128, T_IN], F32)
                nc.scalar.activation(e[:], d[:], Ex, scale=KEXP)
                t = pt.tile([128, T_IN], F32)
                nc.vector.scalar_tensor_tensor(
                    t[:], e[:], 1.0, at[:], sub, mult,
                    accum_out=r1t[:, t_i:t_i + 1])
                tsq = pq.tile([128, T_IN], F32)
                if t_i % 2 == 0:
                    nc.vector.scalar_tensor_tensor(
                        tsq[:], t[:], 0.0, t[:], sub, mult,
                        accum_out=r2t[:, t_i:t_i + 1])
                else:
                    nc.scalar.activation(tsq[:], t[:], Sq,
                                         accum_out=r2t[:, t_i:t_i + 1])
            nc.sync.dma_start(r1p[:], r1t[:])
            nc.sync.dma_start(r2p[:], r2t[:])
    nc.compile()
    return nc


def kernel(att_ws, ilens, olens):
    from concourse.bass_utils import run_bass_kernel_spmd

    att_ws = np.asarray(att_ws)
    ilens_np = np.asarray(ilens).astype(np.int64)
    olens_np = np.asarray(olens).astype(np.int64)
    B, T_out, T_in = att_ws.shape

    ol = olens_np
    il = ilens_np
    R = int(ol.sum())                       # total valid rows
    percore = -(-R // N_CORES)
    T = max(1, -(-percore // 128))
    Npc = T * 128                           # padded rows per core
    N = N_CORES * Npc

    # global row tables
    row_b = np.full(N, -1, np.int64)
    row_x = np.zeros(N, np.int64)
    pos = 0
    for b in range(B):
        o = int(ol[b])
        row_b[pos:pos + o] = b
        row_x[pos:pos + o] = np.arange(o)
        pos += o

    valid = row_b >= 0
    sc_all = np.zeros(N, np.float32)
    bc_all = np.zeros(N, np.float32)
    ilf = il.astype(np.float32)
    olf = ol.astype(np.float32)
    sc_all[valid] = 1.0 / ilf[row_b[valid]]
    bc_all[valid] = -(row_x[valid].astype(np.float32) / olf[row_b[valid]])

    A_all = np.zeros((N, T_in), np.float32)
    pos = 0
    for b in range(B):
        o, i = int(ol[b]), int(il[b])
        A_all[pos:pos + o, :i] = att_ws[b, :o, :i]
        pos += o

    yv = np.broadcast_to(
        np.arange(T_in, dtype=np.float32), (128, T_in)).copy()

    in_maps = []
    for c in range(N_CORES):
        s = c * Npc
        in_maps.append({
            "A": A_all[s:s + Npc],
            "yv": yv,
            "sc": np.ascontiguousarray(sc_all[s:s + Npc].reshape(T, 128).T),
            "bc": np.ascontiguousarray(bc_all[s:s + Npc].reshape(T, 128).T),
        })

    if T not in _cache:
        _cache[T] = _build_program(T)
    nc = _cache[T]
    res = run_bass_kernel_spmd(nc, in_maps, list(range(N_CORES)))

    sum1 = np.zeros(B, np.float64)
    sum2 = np.zeros(B, np.float64)
    for c in range(N_CORES):
        r1 = np.asarray(res.results[c]["r1"], np.float64).T.reshape(-1)
        r2 = np.asarray(res.results[c]["r2"], np.float64).T.reshape(-1)
        s = c * Npc
        vb = row_b[s:s + Npc]
        m = vb >= 0
        np.add.at(sum1, vb[m], r1[m])
        np.add.at(sum2, vb[m], r2[m])

    olf64 = ol.astype(np.float64)
    l1 = (-sum1 / olf64).astype(np.float32)
    l2 = (sum2 / olf64).astype(np.float32)
    return (l1, l2)


# revision 4
# speedup vs baseline: 103724.0093x; 103724.0093x over previous
"""GuidedAttentionLoss on 8 TRN2 cores — 2 width classes (512 / 256).

Rows (b, x) with il_b > 256 are processed at free-width 512; rows with
il_b <= 256 at width 256 (their att is zero beyond il anyway, so the
upper 256 columns would contribute exactly 0 — skip them).  Cuts padded
elements and DMA ~25% vs the single-class version.
"""
import numpy as np

N_CORES = 8
T_IN = 512
KEXP = -3.125

_cache = {}


def _build_program(T512, T256):
    import concourse.bacc as bacc
    import concourse.mybir as mybir
    import concourse.tile as tile

    F32 = mybir.dt.float32
    nc = bacc.Bacc("TRN2", target_bir_lowering=False, debug=False,
                   num_devices=1)
    T = T512 + T256
    A512 = nc.declare_dram_parameter("A512", [max(T512, 1) * 128, 512], F32,
                                     isOutput=False)
    A256 = nc.declare_dram_parameter("A256", [max(T256, 1) * 128, 256], F32,
                                     isOutput=False)
    yvp = nc.declare_dram_parameter("yv", [128, T_IN], F32, isOutput=False)
    scp = nc.declare_dram_parameter("sc", [128, T], F32, isOutput=False)
    bcp = nc.declare_dram_parameter("bc", [128, T], F32, isOutput=False)
    r1p = nc.declare_dram_parameter("r1", [128, T], F32, isOutput=True)
    r2p = nc.declare_dram_parameter("r2", [128, T], F32, isOutput=True)

    Sq = mybir.ActivationFunctionType.Square
    Ex = mybir.ActivationFunctionType.Exp
    sub = mybir.AluOpType.subtract
    mult = mybir.AluOpType.mult

    with tile.TileContext(nc) as tc:
        with tc.tile_pool(name="aux", bufs=1) as aux, \
             tc.tile_pool(name="p", bufs=4) as pa, \
             tc.tile_pool(name="pd", bufs=3) as pd, \
             tc.tile_pool(name="pe", bufs=3) as pe, \
             tc.tile_pool(name="pt", bufs=3) as pt, \
             tc.tile_pool(name="pq", bufs=3) as pq:
            yvt = aux.tile([128, T_IN], F32)
            nc.sync.dma_start(yvt[:], yvp[:])
            sct = aux.tile([128, T], F32)
            nc.sync.dma_start(sct[:], scp[:])
            bct = aux.tile([128, T], F32)
            nc.sync.dma_start(bct[:], bcp[:])
            r1t = aux.tile([128, T], F32)
            r2t = aux.tile([128, T], F32)

            def tile_body(t_i, W, src, row0):
                at = pa.tile([128, W], F32, tag="a")
                nc.sync.dma_start(at[:], src[row0:row0 + 128, :])
                d = pd.tile([128, W], F32, tag="d")
                nc.scalar.activation(d[:], yvt[:, :W], Sq,
                                     bias=bct[:, t_i:t_i + 1],
                                     scale=sct[:, t_i:t_i + 1])
                e = pe.tile([128, W], F32, tag="e")
                nc.scalar.activation(e[:], d[:], Ex, scale=KEXP)
                t = pt.tile([128, W], F32, tag="t")
                nc.vector.scalar_tensor_tensor(
                    t[:], e[:], 1.0, at[:], sub, mult,
                    accum_out=r1t[:, t_i:t_i + 1])
                tsq = pq.tile([128, W], F32, tag="q")
                if t_i % 7 < 5:
                    nc.vector.scalar_tensor_tensor(
                        tsq[:], t[:], 0.0, t[:], sub, mult,
                        accum_out=r2t[:, t_i:t_i + 1])
                else:
                    nc.scalar.activation(tsq[:], t[:], Sq,
                                         accum_out=r2t[:, t_i:t_i + 1])

            for i in range(T512):
                tile_body(i, 512, A512, i * 128)
            for i in range(T256):
                tile_body(T512 + i, 256, A256, i * 128)

            nc.sync.dma_start(r1p[:], r1t[:])
            nc.sync.dma_start(r2p[:], r2t[:])
    nc.compile()
    return nc


def _pack_class(att_ws, il, ol, rows_b, rows_x, n_core_rows, W):
    """rows_b/rows_x: global row lists for this class. Returns per-core
    A arrays + per-core (row_b) map, padded to n_core_rows rows/core."""
    B = att_ws.shape[0]
    nrows = len(rows_b)
    A = np.zeros((N_CORES, n_core_rows, W), np.float32)
    mb = np.full((N_CORES, n_core_rows), -1, np.int64)
    mx = np.zeros((N_CORES, n_core_rows), np.int64)
    # contiguous block split
    base = 0
    for c in range(N_CORES):
        take = min(n_core_rows, max(0, nrows - base))
        if take:
            rb = rows_b[base:base + take]
            rx = rows_x[base:base + take]
            mb[c, :take] = rb
            mx[c, :take] = rx
        base += take
    # fill A row-by-row grouped by sample for speed
    for c in range(N_CORES):
        rb = mb[c]
        rx = mx[c]
        for b in np.unique(rb):
            if b < 0:
                continue
            sel = rb == b
            i = min(int(il[b]), W)
            A[c, sel, :i] = att_ws[b, rx[sel], :i]
    return A, mb, mx


def kernel(att_ws, ilens, olens, _trace=False, _tracedir=None):
    from concourse.bass_utils import run_bass_kernel_spmd

    att_ws = np.asarray(att_ws)
    il = np.asarray(ilens).astype(np.int64)
    ol = np.asarray(olens).astype(np.int64)
    B, T_out, T_in = att_ws.shape

    big = il > 256          # class-512 samples
    rb_l, rx_l = [], []
    for cls in (True, False):
        sel = np.nonzero(big == cls)[0]
        rb = np.repeat(sel, ol[sel])
        rx = np.concatenate([np.arange(int(ol[b])) for b in sel]) \
            if len(sel) else np.zeros(0, np.int64)
        rb_l.append(rb)
        rx_l.append(rx)

    R512, R256 = len(rb_l[0]), len(rb_l[1])
    T512 = -(-(-(-R512 // N_CORES)) // 128) if R512 else 0
    T256 = -(-(-(-R256 // N_CORES)) // 128) if R256 else 0
    T = T512 + T256

    A5, mb5, mx5 = _pack_class(att_ws, il, ol, rb_l[0], rx_l[0],
                               T512 * 128 if T512 else 0, 512)
    A2, mb2, mx2 = _pack_class(att_ws, il, ol, rb_l[1], rx_l[1],
                               T256 * 128 if T256 else 0, 256)

    ilf = il.astype(np.float32)
    olf = ol.astype(np.float64)
    yv = np.broadcast_to(np.arange(T_IN, dtype=np.float32),
                         (128, T_IN)).copy()

    in_maps = []
    maps = []
    for c in range(N_CORES):
        mb = np.concatenate([mb5[c] if T512 else np.zeros(0, np.int64),
                             mb2[c] if T256 else np.zeros(0, np.int64)])
        mx = np.concatenate([mx5[c] if T512 else np.zeros(0, np.int64),
                             mx2[c] if T256 else np.zeros(0, np.int64)])
        v = mb >= 0
        sc = np.zeros(len(mb), np.float32)
        bc = np.zeros(len(mb), np.float32)
        sc[v] = 1.0 / ilf[mb[v]]
        bc[v] = -(mx[v] / olf[mb[v]]).astype(np.float32)
        in_maps.append({
            "A512": A5[c] if T512 else np.zeros((128, 512), np.float32),
            "A256": A2[c] if T256 else np.zeros((128, 256), np.float32),
            "yv": yv,
            "sc": np.ascontiguousarray(sc.reshape(T, 128).T),
            "bc": np.ascontiguousarray(bc.reshape(T, 128).T),
        })
        maps.append(mb)

    key = (T512, T256)
    if key not in _cache:
        _cache[key] = _build_program(T512, T256)
    nc = _cache[key]
    kw = {}
    if _trace:
        kw = dict(trace=True, tmpdir=_tracedir)
    res = run_bass_kernel_spmd(nc, in_maps, list(range(N_CORES)), **kw)
    kernel._last_exec_ns = getattr(res, "exec_time_ns", None)

    sum1 = np.zeros(B, np.float64)
    sum2 = np.zeros(B, np.float64)
    for c in range(N_CORES):
        r1 = np.asarray(res.results[c]["r1"], np.float64).T.reshape(-1)
        r2 = np.asarray(res.results[c]["r2"], np.float64).T.reshape(-1)
        mb = maps[c]
        m = mb >= 0
        np.add.at(sum1, mb[m], r1[m])
        np.add.at(sum2, mb[m], r2[m])

    l1 = (-sum1 / olf).astype(np.float32)
    l2 = (sum2 / olf).astype(np.float32)
    return (l1, l2)
